# revision 6
# baseline (speedup 1.0000x reference)
"""Trainium2 Bass kernel for BaseLUTLayer (probabilistic LUT node eval).

Math (per reference):
  x_eff = where(flip, 1 - x, x)                      # (B, IN)
  g[b,n,j] = x_eff[b, mapping[n,j]]                  # gather, (B, N, 6)
  out[b,n] = sum_k sigmoid(lut[n,k]) * prod_j (g_j if bit_j(k) else 1-g_j)

Host prep folds the flip into x (pure input re-encoding, like the
transpose/fp16 cast) and ships the sigmoid'd LUT in its Moebius
(iterated-difference) basis c, so the device evaluates the multilinear
polynomial with pure FMA folds:
  U[i]   = c[2i] + a0 * c[2i+1]          (per-partition-scalar FMA, 32x)
  Q_j[m] = Q[2m] + a_j * Q[2m+1]         (tensor mul+add, levels 1..5)

Sharding: nodes split 8 ways (1024 nodes/core); batch replicated.
x_eff is host-transposed to (IN, B) fp16 so dma_gather can fetch one
512B row per (node, fanin) index.  Per-core output is (1024, 256),
host concatenates + transposes.

Engine balance per tile: the 32 bottom FMAs split ACT/DVE (ACT does
act(Identity, scale, bias); DVE tensor_scalar runs in 4x mode), the
fold levels alternate DVE (L1/L3/L5, 2x tensor_tensor) and Pool
(L2/L4 via scalar_tensor_tensor with op0=bypass, which the cost model
rates at 0.60 efficiency vs 0.42 for plain tensor_tensor).
"""

import numpy as np

B = 256
IN = 8192
NN = 8192
FAN = 6
NPAT = 64
NCORES = 8
PT = 128  # nodes per tile (partition dim)

_CACHE = {}

# bottom-fold engine split: i -> "act" | "dve" | "pool"
_TS_HOMES = ["pool"] * 5 + ["dve"] * 1 + ["act"] * 26
# fold-level engine owners (1..5): "dve" or "pool"
_LEVEL_HOMES = {1: "dve", 2: "dve", 3: "pool", 4: "dve", 5: "pool"}


def _build_nc(nl, b, inp, fp16=True):
    """Build + compile the SPMD Bass program for one core's slice.

    nl: nodes per core, b: batch (replicated), inp: input size.
    """
    import concourse.bacc as bacc
    import concourse.mybir as mybir
    from concourse.tile import TileContext
    from concourse._compat import get_trn_type

    dt = mybir.dt
    Alu = mybir.AluOpType
    Act = mybir.ActivationFunctionType

    nt = nl // PT
    n_idx = nl * FAN          # gather indices total
    n_idx_t = PT * FAN        # per tile (768)
    iw = n_idx // 16          # idx wrap columns

    nc = bacc.Bacc(
        get_trn_type() or "TRN2",
        target_bir_lowering=False,
        debug=False,
        num_devices=NCORES,
    )
    rowb = 2 * b              # fp16 x_eff row bytes
    xT = nc.dram_tensor("xfT", [inp, rowb], dt.uint8, kind="ExternalInput")
    ctab = nc.dram_tensor("lut", [nl, NPAT], dt.float32, kind="ExternalInput")
    idx = nc.dram_tensor("idx", [128, iw], dt.int16, kind="ExternalInput")
    outT = nc.dram_tensor("outT", [nl, b], dt.float32, kind="ExternalOutput")

    cdt = dt.float16 if fp16 else dt.float32

    def eng(name):
        return nc.vector if name == "dve" else nc.gpsimd

    with TileContext(nc) as tc:
        with (
            tc.tile_pool(name="const", bufs=1) as cpool,
            tc.tile_pool(name="ld", bufs=2) as ld,
            tc.tile_pool(name="work", bufs=2) as wk,
        ):
            idx_sb = cpool.tile([128, iw], dt.int16)
            nc.sync.dma_start(idx_sb[:, :], idx[:, :])

            for t in range(nt):
                # --- loads: gather 6 x_eff rows per node + c table slice ---
                g = ld.tile([128, FAN, rowb], dt.uint8, tag="g")
                nc.gpsimd.dma_gather(
                    g[:, :, :], xT[:, :],
                    idx_sb[:, t * (n_idx_t // 16):(t + 1) * (n_idx_t // 16)],
                    n_idx_t, n_idx_t, rowb,
                )
                xg = g[:, :, :].bitcast(dt.float16)  # [128, 6, b]
                c = ld.tile([128, NPAT], dt.float32, tag="c")
                nc.sync.dma_start(c[:, :], ctab[t * PT:(t + 1) * PT, :])

                a = [xg[:, j, :] for j in range(FAN)]

                # --- bottom fold (bit 0): U[i] = c[2i] + a0*c[2i+1] ---
                U = wk.tile([128, 32, b], cdt, tag="U")
                for i in range(32):
                    sc = c[:, 2 * i + 1:2 * i + 2]
                    bi = c[:, 2 * i:2 * i + 1]
                    h = _TS_HOMES[i]
                    if h == "act":
                        nc.scalar.activation(
                            U[:, i, :], a[0], Act.Identity, scale=sc, bias=bi)
                    elif h == "pool":
                        nc.gpsimd.tensor_scalar(
                            out=U[:, i, :], in0=a[0], scalar1=sc, scalar2=bi,
                            op0=Alu.mult, op1=Alu.add)
                    else:
                        nc.vector.tensor_scalar(
                            out=U[:, i, :], in0=a[0], scalar1=sc, scalar2=bi,
                            op0=Alu.mult, op1=Alu.add)

                # --- fold levels 1..5: V[m] = V[2m] + a_j * V[2m+1] ---
                V = U
                for j in range(1, 6):
                    h = 32 >> j
                    e = eng(_LEVEL_HOMES[j])
                    ab = xg[:, j:j + 1, :].broadcast_to([128, h, b])
                    P = wk.tile([128, h, b], cdt, tag=f"P{j}")
                    e.tensor_mul(P[:, :, :], V[:, 1::2, :], ab)
                    odt = dt.float32 if j == 5 else cdt
                    Vn = wk.tile([128, h, b], odt, tag=f"V{j}")
                    e.tensor_add(Vn[:, :, :], P[:, :, :], V[:, 0::2, :])
                    V = Vn

                nc.sync.dma_start(outT[t * PT:(t + 1) * PT, :], V[:, 0, :])

    nc.compile()
    return nc


def _prep_core_inputs(x, lut_table, mapping, flip_mask, nl, b, inp, n_cores=NCORES):
    """Host-side input prep: flip fold, fp16 transpose, sigmoid+Moebius table,
    gather-index packing."""
    x = np.asarray(x)
    flip = np.asarray(flip_mask)
    x_eff = np.where(flip, 1.0 - x, x).astype(np.float16)
    xf = np.ascontiguousarray(x_eff.T).view(np.uint8)              # (IN, 2B)

    sig = 1.0 / (1.0 + np.exp(-np.asarray(lut_table, dtype=np.float32)))
    c = sig.copy()
    nn = c.shape[0]
    for j in range(6):
        v = c.reshape(nn, 2 ** (5 - j), 2, 2 ** j)
        v[:, :, 1, :] -= v[:, :, 0, :]

    nt = nl // PT
    in_maps = []
    for ci in range(n_cores):
        sl = slice(ci * nl, (ci + 1) * nl)
        c_c = np.ascontiguousarray(c[sl])
        m_c = np.asarray(mapping[sl])                              # (nl, 6) int32
        # gather order: j = (t*6+f)*128 + p  ->  m_c[t*128+p, f]
        order = m_c.reshape(nt, PT, FAN).transpose(0, 2, 1).reshape(-1)
        idx16 = order.astype(np.int16)
        wrapped = np.ascontiguousarray(idx16.reshape(-1, 16).T)    # (16, nl*6/16)
        idx_full = np.tile(wrapped, (8, 1))                        # (128, ...)
        in_maps.append({"xfT": xf, "lut": c_c, "idx": idx_full})
    return in_maps


def _run(nc, in_maps, **kw):
    from concourse.bass_utils import run_bass_kernel_spmd

    last = None
    for attempt in range(3):
        try:
            return run_bass_kernel_spmd(nc, in_maps, list(range(NCORES)), **kw)
        except Exception as e:  # transient device errors happen on this fabric
            last = e
            if "UNRECOVERABLE" not in str(e) and "UNAVAILABLE" not in str(e):
                raise
    raise last


def kernel(x, lut_table, mapping, flip_mask):
    b, inp = x.shape
    nn = lut_table.shape[0]
    nl = nn // NCORES
    key = (nl, b, inp)
    if key not in _CACHE:
        _CACHE[key] = _build_nc(nl, b, inp)
    nc = _CACHE[key]
    in_maps = _prep_core_inputs(x, lut_table, mapping, flip_mask, nl, b, inp)
    res = _run(nc, in_maps)
    outT = np.concatenate([res.results[c]["outT"] for c in range(NCORES)], axis=0)
    return np.ascontiguousarray(outT.T, dtype=np.float32)


# revision 8
# speedup vs baseline: 1.2599x; 1.2599x over previous
"""Trainium2 Bass kernel for BaseLUTLayer (probabilistic LUT node eval).

Math (per reference):
  x_eff = where(flip, 1 - x, x)                      # (B, IN)
  g[b,n,j] = x_eff[b, mapping[n,j]]                  # gather, (B, N, 6)
  out[b,n] = sum_k sigmoid(lut[n,k]) * prod_j (g_j if bit_j(k) else 1-g_j)

Host prep folds the flip into x (pure input re-encoding, like the
transpose/fp16 cast) and ships the sigmoid'd LUT in its Moebius
(iterated-difference) basis c, so the device evaluates the multilinear
polynomial with pure FMA folds:
  U[i]   = c[2i] + a0 * c[2i+1]          (per-partition-scalar FMA, 32x)
  Q_j[m] = Q[2m] + a_j * Q[2m+1]         (tensor mul+add, levels 1..5)

Sharding: nodes split 8 ways (1024 nodes/core); batch replicated.
x_eff is host-transposed to (IN, B) fp16 so dma_gather can fetch one
512B row per (node, fanin) index.  Per-core output is (1024, 256),
host concatenates + transposes.

Engine balance per tile: the 32 bottom FMAs split ACT/DVE (ACT does
act(Identity, scale, bias); DVE tensor_scalar runs in 4x mode), the
fold levels alternate DVE (L1/L3/L5, 2x tensor_tensor) and Pool
(L2/L4 via scalar_tensor_tensor with op0=bypass, which the cost model
rates at 0.60 efficiency vs 0.42 for plain tensor_tensor).
"""

import numpy as np

B = 256
IN = 8192
NN = 8192
FAN = 6
NPAT = 64
NCORES = 8
PT = 128  # nodes per tile (partition dim)

_CACHE = {}

# bottom-fold engine split: i -> "act" | "dve" | "pool"
_TS_HOMES = (["pool"] * 5 + ["act"] * 10 + ["dve"] * 2 +
             ["pool"] * 4 + ["act"] * 10 + ["dve"] * 1)
# fold-level engine owners (1..5): "dve" or "pool"
_LEVEL_HOMES = {1: "dve", 2: "dve", 3: "pool", 4: "dve", 5: "dve"}


def _build_nc(nl, b, inp, fp16=True):
    """Build + compile the SPMD Bass program for one core's slice.

    nl: nodes per core, b: batch (replicated), inp: input size.
    """
    import concourse.bacc as bacc
    import concourse.mybir as mybir
    from concourse.tile import TileContext
    from concourse._compat import get_trn_type

    dt = mybir.dt
    Alu = mybir.AluOpType
    Act = mybir.ActivationFunctionType

    nt = nl // PT
    n_idx = nl * FAN          # gather indices total
    n_idx_t = PT * FAN        # per tile (768)
    iw = n_idx // 16          # idx wrap columns

    nc = bacc.Bacc(
        get_trn_type() or "TRN2",
        target_bir_lowering=False,
        debug=False,
        num_devices=NCORES,
    )
    rowb = 2 * b              # fp16 x_eff row bytes
    xT = nc.dram_tensor("xfT", [inp, rowb], dt.uint8, kind="ExternalInput")
    ctab = nc.dram_tensor("lut", [nl, NPAT], dt.float32, kind="ExternalInput")
    idx = nc.dram_tensor("idx", [128, iw], dt.int16, kind="ExternalInput")
    outT = nc.dram_tensor("outT", [nl, b], dt.float32, kind="ExternalOutput")

    cdt = dt.float16 if fp16 else dt.float32

    def eng(name):
        return nc.vector if name == "dve" else nc.gpsimd

    with TileContext(nc) as tc:
        with (
            tc.tile_pool(name="const", bufs=1) as cpool,
            tc.tile_pool(name="ld", bufs=3) as ld,
            tc.tile_pool(name="us", bufs=3) as us,
            tc.tile_pool(name="work", bufs=2) as wk,
        ):
            idx_sb = cpool.tile([128, iw], dt.int16)
            nc.sync.dma_start(idx_sb[:, :], idx[:, :])

            def issue_loads(t):
                g = ld.tile([128, FAN, rowb], dt.uint8, tag="g")
                nc.gpsimd.dma_gather(
                    g[:, :, :], xT[:, :],
                    idx_sb[:, t * (n_idx_t // 16):(t + 1) * (n_idx_t // 16)],
                    n_idx_t, n_idx_t, rowb,
                )
                xg = g[:, :, :].bitcast(dt.float16)  # [128, 6, b]
                c = ld.tile([128, NPAT], dt.float32, tag="c")
                nc.sync.dma_start(c[:, :], ctab[t * PT:(t + 1) * PT, :])
                return xg, c

            def issue_bottom(xg, c):
                # U[i] = c[2i] + a0*c[2i+1]
                a0 = xg[:, 0, :]
                U = us.tile([128, 32, b], cdt, tag="U")
                for i in range(32):
                    sc = c[:, 2 * i + 1:2 * i + 2]
                    bi = c[:, 2 * i:2 * i + 1]
                    h = _TS_HOMES[i]
                    if h == "act":
                        nc.scalar.activation(
                            U[:, i, :], a0, Act.Identity, scale=sc, bias=bi)
                    elif h == "pool":
                        nc.gpsimd.tensor_scalar(
                            out=U[:, i, :], in0=a0, scalar1=sc, scalar2=bi,
                            op0=Alu.mult, op1=Alu.add)
                    else:
                        nc.vector.tensor_scalar(
                            out=U[:, i, :], in0=a0, scalar1=sc, scalar2=bi,
                            op0=Alu.mult, op1=Alu.add)
                return U

            def issue_folds(t, xg, U):
                # fold levels 1..5: V[m] = V[2m] + a_j * V[2m+1]
                V = U
                for j in range(1, 6):
                    h = 32 >> j
                    e = eng(_LEVEL_HOMES[j])
                    ab = xg[:, j:j + 1, :].broadcast_to([128, h, b])
                    P = wk.tile([128, h, b], cdt, tag=f"P{j}")
                    e.tensor_mul(P[:, :, :], V[:, 1::2, :], ab)
                    odt = dt.float32 if j == 5 else cdt
                    Vn = wk.tile([128, h, b], odt, tag=f"V{j}")
                    e.tensor_add(Vn[:, :, :], P[:, :, :], V[:, 0::2, :])
                    V = Vn
                nc.sync.dma_start(outT[t * PT:(t + 1) * PT, :], V[:, 0, :])

            # software pipeline, skewed one tile: loads+bottom for tile t are
            # issued before the folds of tile t-1 so every engine queue holds
            # independent work while the serial fold chain of the prior tile
            # drains.
            pend = None  # (t, xg, U)
            for t in range(nt):
                xg, c = issue_loads(t)
                U = issue_bottom(xg, c)
                if pend is not None:
                    issue_folds(*pend)
                pend = (t, xg, U)
            issue_folds(*pend)

    nc.compile()
    return nc


def _prep_core_inputs(x, lut_table, mapping, flip_mask, nl, b, inp, n_cores=NCORES):
    """Host-side input prep: flip fold, fp16 transpose, sigmoid+Moebius table,
    gather-index packing."""
    x = np.asarray(x)
    flip = np.asarray(flip_mask)
    x_eff = np.where(flip, 1.0 - x, x).astype(np.float16)
    xf = np.ascontiguousarray(x_eff.T).view(np.uint8)              # (IN, 2B)

    sig = 1.0 / (1.0 + np.exp(-np.asarray(lut_table, dtype=np.float32)))
    c = sig.copy()
    nn = c.shape[0]
    for j in range(6):
        v = c.reshape(nn, 2 ** (5 - j), 2, 2 ** j)
        v[:, :, 1, :] -= v[:, :, 0, :]

    nt = nl // PT
    in_maps = []
    for ci in range(n_cores):
        sl = slice(ci * nl, (ci + 1) * nl)
        c_c = np.ascontiguousarray(c[sl])
        m_c = np.asarray(mapping[sl])                              # (nl, 6) int32
        # gather order: j = (t*6+f)*128 + p  ->  m_c[t*128+p, f]
        order = m_c.reshape(nt, PT, FAN).transpose(0, 2, 1).reshape(-1)
        idx16 = order.astype(np.int16)
        wrapped = np.ascontiguousarray(idx16.reshape(-1, 16).T)    # (16, nl*6/16)
        idx_full = np.tile(wrapped, (8, 1))                        # (128, ...)
        in_maps.append({"xfT": xf, "lut": c_c, "idx": idx_full})
    return in_maps


def _run(nc, in_maps, **kw):
    from concourse.bass_utils import run_bass_kernel_spmd

    last = None
    for attempt in range(3):
        try:
            return run_bass_kernel_spmd(nc, in_maps, list(range(NCORES)), **kw)
        except Exception as e:  # transient device errors happen on this fabric
            last = e
            if "UNRECOVERABLE" not in str(e) and "UNAVAILABLE" not in str(e):
                raise
    raise last


def kernel(x, lut_table, mapping, flip_mask):
    b, inp = x.shape
    nn = lut_table.shape[0]
    nl = nn // NCORES
    key = (nl, b, inp)
    if key not in _CACHE:
        _CACHE[key] = _build_nc(nl, b, inp)
    nc = _CACHE[key]
    in_maps = _prep_core_inputs(x, lut_table, mapping, flip_mask, nl, b, inp)
    res = _run(nc, in_maps)
    outT = np.concatenate([res.results[c]["outT"] for c in range(NCORES)], axis=0)
    return np.ascontiguousarray(outT.T, dtype=np.float32)


# revision 11
# speedup vs baseline: 1.5031x; 1.1930x over previous
"""Trainium2 Bass kernel for BaseLUTLayer (probabilistic LUT node eval).

Math (per reference):
  x_eff = where(flip, 1 - x, x)                      # (B, IN)
  g[b,n,j] = x_eff[b, mapping[n,j]]                  # gather, (B, N, 6)
  out[b,n] = sum_k sigmoid(lut[n,k]) * prod_j (g_j if bit_j(k) else 1-g_j)

Host prep folds the flip into x (pure input re-encoding, like the
transpose/fp16 cast) and ships the sigmoid'd LUT in its Moebius
(iterated-difference) basis c, so the device evaluates the multilinear
polynomial with pure FMA folds:
  U[i]   = c[2i] + a0 * c[2i+1]          (per-partition-scalar FMA, 32x)
  Q_j[m] = Q[2m] + a_j * Q[2m+1]         (tensor mul+add, levels 1..5)

Sharding: nodes split 8 ways (1024 nodes/core); batch replicated.
x_eff is host-transposed to (IN, B) fp16 so dma_gather can fetch one
512B row per (node, fanin) index.  Per-core output is (1024, 256),
host concatenates + transposes.

Engine balance per tile: the 32 bottom FMAs split ACT/DVE (ACT does
act(Identity, scale, bias); DVE tensor_scalar runs in 4x mode), the
fold levels alternate DVE (L1/L3/L5, 2x tensor_tensor) and Pool
(L2/L4 via scalar_tensor_tensor with op0=bypass, which the cost model
rates at 0.60 efficiency vs 0.42 for plain tensor_tensor).
"""

import numpy as np

B = 256
IN = 8192
NN = 8192
FAN = 6
NPAT = 64
NCORES = 8
PT = 128  # nodes per tile (partition dim)

_CACHE = {}

# bottom-fold engine split: i -> "act" | "dve" | "pool"
_TS_HOMES = (["pool"] * 4 + ["act"] * 8 + ["dve"] * 4 +
             ["pool"] * 4 + ["act"] * 8 + ["dve"] * 4)
# batch-column split for fold levels 3-5: cols [0:_S3] on DVE, [_S3:b] on Pool
_S3 = 128


def _build_nc(nl, b, inp, fp16=True):
    """Build + compile the SPMD Bass program for one core's slice.

    nl: nodes per core, b: batch (replicated), inp: input size.
    """
    import concourse.bacc as bacc
    import concourse.mybir as mybir
    from concourse.tile import TileContext
    from concourse._compat import get_trn_type

    dt = mybir.dt
    Alu = mybir.AluOpType
    Act = mybir.ActivationFunctionType

    nt = nl // PT
    n_idx = nl * FAN          # gather indices total
    n_idx_t = PT * FAN        # per tile (768)
    iw = n_idx // 16          # idx wrap columns

    nc = bacc.Bacc(
        get_trn_type() or "TRN2",
        target_bir_lowering=False,
        debug=False,
        num_devices=NCORES,
    )
    rowb = 2 * b              # fp16 x_eff row bytes
    xT = nc.dram_tensor("xfT", [inp, rowb], dt.uint8, kind="ExternalInput")
    ctab = nc.dram_tensor("lut", [nl, NPAT], dt.float32, kind="ExternalInput")
    idx = nc.dram_tensor("idx", [128, iw], dt.int16, kind="ExternalInput")
    outT = nc.dram_tensor("outT", [nl, b], dt.float32, kind="ExternalOutput")

    cdt = dt.float16 if fp16 else dt.float32

    def eng(name):
        return nc.vector if name == "dve" else nc.gpsimd

    with TileContext(nc) as tc:
        with (
            tc.tile_pool(name="const", bufs=1) as cpool,
            tc.tile_pool(name="ld", bufs=5) as ld,
            tc.tile_pool(name="us", bufs=3) as us,
            tc.tile_pool(name="work", bufs=3) as wk,
        ):
            idx_sb = cpool.tile([128, iw], dt.int16)
            nc.sync.dma_start(idx_sb[:, :], idx[:, :])

            def issue_loads(t):
                g = ld.tile([128, FAN, rowb], dt.uint8, tag="g")
                nc.gpsimd.dma_gather(
                    g[:, :, :], xT[:, :],
                    idx_sb[:, t * (n_idx_t // 16):(t + 1) * (n_idx_t // 16)],
                    n_idx_t, n_idx_t, rowb,
                )
                xg = g[:, :, :].bitcast(dt.float16)  # [128, 6, b]
                c = ld.tile([128, NPAT], dt.float32, tag="c")
                nc.sync.dma_start(c[:, :], ctab[t * PT:(t + 1) * PT, :])
                return xg, c

            def issue_bottom(xg, c):
                # U[i] = c[2i] + a0*c[2i+1]
                a0 = xg[:, 0, :]
                U = us.tile([128, 32, b], cdt, tag="U")
                for i in range(32):
                    sc = c[:, 2 * i + 1:2 * i + 2]
                    bi = c[:, 2 * i:2 * i + 1]
                    h = _TS_HOMES[i]
                    if h == "act":
                        nc.scalar.activation(
                            U[:, i, :], a0, Act.Identity, scale=sc, bias=bi)
                    elif h == "pool":
                        nc.gpsimd.tensor_scalar(
                            out=U[:, i, :], in0=a0, scalar1=sc, scalar2=bi,
                            op0=Alu.mult, op1=Alu.add)
                    else:
                        nc.vector.tensor_scalar(
                            out=U[:, i, :], in0=a0, scalar1=sc, scalar2=bi,
                            op0=Alu.mult, op1=Alu.add)
                return U

            def issue_l12(xg, U):
                # fold levels 1-2 on DVE, full width
                V = U
                for j in (1, 2):
                    h = 32 >> j
                    ab = xg[:, j:j + 1, :].broadcast_to([128, h, b])
                    P = wk.tile([128, h, b], cdt, tag=f"P{j}")
                    nc.vector.tensor_mul(P[:, :, :], V[:, 1::2, :], ab)
                    Vn = wk.tile([128, h, b], cdt, tag=f"V{j}")
                    nc.vector.tensor_add(Vn[:, :, :], P[:, :, :], V[:, 0::2, :])
                    V = Vn
                return V

            def issue_tail(t, xg, V2, e, c0, c1, pv):
                # fold levels 3-5 on cols [c0:c1], engine e, independent chain
                V = V2
                w = c1 - c0
                for j in (3, 4, 5):
                    h = 32 >> j
                    ab = xg[:, j:j + 1, c0:c1].broadcast_to([128, h, w])
                    P = pv.tile([128, h, w], cdt, tag=f"P{j}_{c0}")
                    e.tensor_mul(P[:, :, :], V[:, 1::2, c0:c1] if j == 3 else V[:, 1::2, :], ab)
                    odt = dt.float32 if j == 5 else cdt
                    Vn = pv.tile([128, h, w], odt, tag=f"V{j}_{c0}")
                    e.tensor_add(Vn[:, :, :], P[:, :, :], V[:, 0::2, c0:c1] if j == 3 else V[:, 0::2, :])
                    V = Vn
                nc.sync.dma_start(outT[t * PT:(t + 1) * PT, c0:c1], V[:, 0, :])

            # software pipeline: loads/bottom for tile t, L1-2 folds for t-2,
            # tail folds for t-3 (DVE cols [0:_S3] and Pool cols [_S3:b] run
            # as independent chains), so no engine queue ever head-blocks on
            # another engine's in-flight work.
            st = {}  # t -> dict with xg, U, V2
            for t in range(nt + 3):
                if t < nt:
                    xg, c = issue_loads(t)
                    U = issue_bottom(xg, c)
                    st[t] = {"xg": xg, "U": U}
                if t - 2 >= 0 and t - 2 < nt:
                    s2 = st[t - 2]
                    s2["V2"] = issue_l12(s2["xg"], s2["U"])
                if t - 3 >= 0:
                    s3 = st.pop(t - 3)
                    issue_tail(t - 3, s3["xg"], s3["V2"], nc.vector, 0, _S3, wk)
                    issue_tail(t - 3, s3["xg"], s3["V2"], nc.gpsimd, _S3, b, wk)

    nc.compile()
    return nc


def _prep_core_inputs(x, lut_table, mapping, flip_mask, nl, b, inp, n_cores=NCORES):
    """Host-side input prep: flip fold, fp16 transpose, sigmoid+Moebius table,
    gather-index packing."""
    x = np.asarray(x)
    flip = np.asarray(flip_mask)
    x_eff = np.where(flip, 1.0 - x, x).astype(np.float16)
    xf = np.ascontiguousarray(x_eff.T).view(np.uint8)              # (IN, 2B)

    sig = 1.0 / (1.0 + np.exp(-np.asarray(lut_table, dtype=np.float32)))
    c = sig.copy()
    nn = c.shape[0]
    for j in range(6):
        v = c.reshape(nn, 2 ** (5 - j), 2, 2 ** j)
        v[:, :, 1, :] -= v[:, :, 0, :]

    nt = nl // PT
    in_maps = []
    for ci in range(n_cores):
        sl = slice(ci * nl, (ci + 1) * nl)
        c_c = np.ascontiguousarray(c[sl])
        m_c = np.asarray(mapping[sl])                              # (nl, 6) int32
        # gather order: j = (t*6+f)*128 + p  ->  m_c[t*128+p, f]
        order = m_c.reshape(nt, PT, FAN).transpose(0, 2, 1).reshape(-1)
        idx16 = order.astype(np.int16)
        wrapped = np.ascontiguousarray(idx16.reshape(-1, 16).T)    # (16, nl*6/16)
        idx_full = np.tile(wrapped, (8, 1))                        # (128, ...)
        in_maps.append({"xfT": xf, "lut": c_c, "idx": idx_full})
    return in_maps


def _run(nc, in_maps, **kw):
    from concourse.bass_utils import run_bass_kernel_spmd

    last = None
    for attempt in range(3):
        try:
            return run_bass_kernel_spmd(nc, in_maps, list(range(NCORES)), **kw)
        except Exception as e:  # transient device errors happen on this fabric
            last = e
            if "UNRECOVERABLE" not in str(e) and "UNAVAILABLE" not in str(e):
                raise
    raise last


def kernel(x, lut_table, mapping, flip_mask):
    b, inp = x.shape
    nn = lut_table.shape[0]
    nl = nn // NCORES
    key = (nl, b, inp)
    if key not in _CACHE:
        _CACHE[key] = _build_nc(nl, b, inp)
    nc = _CACHE[key]
    in_maps = _prep_core_inputs(x, lut_table, mapping, flip_mask, nl, b, inp)
    res = _run(nc, in_maps)
    outT = np.concatenate([res.results[c]["outT"] for c in range(NCORES)], axis=0)
    return np.ascontiguousarray(outT.T, dtype=np.float32)
